# revision 4
# baseline (speedup 1.0000x reference)
"""Single-head causal attention (B=8, T=2048, C=1024, H=64) on 8 NeuronCores.

Data-parallel over batch: one batch element per core. The per-core kernel is
built around three ideas: 16-bit/8-bit matmul operands (1 cycle/row at any
moving-dim size), minimal PSUM-exit traffic (only ACT and DVE can read PSUM;
gpsimd cannot), and deep pipelining of the PE -> exp-engine -> PE chains.

  - x is streamed as fp8e4m3 xT plus an fp8 residual (x8 + r8), same bytes
    as bf16 but enabling DoubleRow matmuls (0.5 cycles/row, K=256/step).
  - q|k projection: one fused DoubleRow pass, psum = [Wk8|Wq8]^T @ x8.
    ACT exits k with the 1/sqrt(C) scale folded in; DVE exits q with the
    partition shift 64:127 -> 0:63 and +bq folded in. bk is dropped
    entirely (a constant along a score row is softmax-invariant).
    Projection error is irrelevant downstream: scores are ~N(0, 0.083), so
    a few-percent q/k error perturbs softmax weights by only ~0.4%.
  - v needs full precision (its error hits the output directly): computed
    in the [t, h] orientation as three DoubleRow passes
    x8@Wv8 + x8@Wvr8 + r8@Wv8, where Wvr8 is the Wv quantization residual
    stored as fp8e5m2 (e4m3 subnormals cannot represent it). This captures
    x and Wv to second order (~0.2% error) at fp8 speed. No PE transposes
    anywhere in the kernel.
  - scores computed transposed, s^T[kv, q], into rotating 2-bank PSUM pair
    tiles ([128, 2, 2, 256]: two kv-chunk pairs x 256-q super-chunk) drawn
    from two single-buf pools plus a separate single-tile pool, so a new
    score tile never waits on the immediately preceding exp op (the Tile
    framework coarsens subtile deps on one memref, so distinct pool tiles
    are load-bearing here).
  - softmax exp is split across ACT and DVE, strictly alternating: ACT does
    exact exp (PSUM -> fp16); DVE does a 1-op Schraudolph approximation
    (tensor_scalar mult+add -> int16, bitcast fp16:
    i16 = 1024/ln2 * s + 15*1024 - 44, ~1% rel err). Tiles containing
    diagonal blocks always go to ACT: their rows have few softmax terms and
    correlated Schraudolph error there doubles the end-to-end error.
  - causal masking: no masking in the score path at all. The idle gpsimd
    engine zeroes the above-diagonal probabilities in SBUF with a
    tri-mask multiply after exp (it cannot touch PSUM, but this is pure
    SBUF work).
  - PV in the out[q, h] orientation: lhsT = fp16 probability block
    (stationary), rhs = [v | 1] fp16 (65 wide) so the softmax denominator
    accumulates in column 64 for free. One PSUM bank holds 4 q-chunks'
    accumulators; accumulation groups per bank are strictly sequential
    (the interpreter pending-zero model resets whole 2KB banks on
    start_tensor_calc). PV for super-chunk I is emitted one super-chunk
    behind its scores/exp so the PE never waits on the exp engines.
  - out exit: one DVE copy [128, 4, 65] PSUM -> SBUF, DMA to HBM; the
    division by the denominator and +bv happen on the host (softmax rows
    sum to one, so out = wei@v_raw/den + bv).
  - DMA: all input DMAs issue from the SP queue (keeping the ACT sequencer
    free for psum exits), ordered so the first projection's weights and
    x-slice land first; out DMAs ride the gpsimd SWDGE queue.

This walrus build accepts only ONE sync-wait command per instruction; Tile
can emit several. `_split_waits` rewrites the scheduled module, moving
excess waits onto injected NoOps on the same engine sequencer.
"""

import numpy as np
import ml_dtypes

import concourse.bass as bass
import concourse.mybir as mybir
from concourse.tile import TileContext
from concourse.bass_utils import run_bass_kernel_spmd

F32 = mybir.dt.float32
BF16 = mybir.dt.bfloat16
F16 = mybir.dt.float16
FP8 = mybir.dt.float8e4
FP8E5 = mybir.dt.float8e5
I16 = mybir.dt.int16
DR = mybir.MatmulPerfMode.DoubleRow
AF = mybir.ActivationFunctionType
ALU = mybir.AluOpType

import os
ABL = set(os.environ.get("KABL", "").split(","))
KEXP = os.environ.get("KEXP", "altv")   # greedy | alt | altv
KLAG = os.environ.get("KLAG", "last")     # prop | first | last
KOUT = os.environ.get("KOUT", "a")        # v | a  (out-copy engine)

B, T, C, H = 8, 2048, 1024, 64
NCORES = 8
NCC = C // 128            # 8 contraction chunks
NQC = T // 128            # 16 q chunks
NSUP = NQC // 2           # 8 q super-chunks of 256
NSL = T // 512            # 4 t slices
SCALE = 1.0 / np.sqrt(C)  # 1/32

# fp16 Schraudolph exp: bitcast16(int16(A*s + B)) ~= e^s for |s| < ~1
EXP_A = 1024.0 / np.log(2.0)
EXP_B = 15.0 * 1024.0 - 44.0

# setup tensor column layout (bf16)
TRI0 = 0                   # [128, 128] causal 0/1 mask (kv <= q), read as fp16
S_COLS = TRI0 + 128
# fp8 setup tensor: DoubleRow weight layouts [p, d, u, m]
W8QK0 = 0                  # [128, 4, 2, 128] of [Wk | Wq]
W8V0 = W8QK0 + 1024        # [128, 4, 2, 64] of Wv
S8_COLS = W8V0 + 512
# separate e5m2 tensor: Wv residual is ~1e-3 scale, below e4m3's subnormal
# floor; e5m2's wider exponent range represents it fine


def _split_waits(nc, max_waits=1):
    n_split = 0
    for f in nc.m.functions:
        for blk in f.blocks:
            out = []
            changed = False
            for inst in blk.instructions:
                si = inst.sync_info
                if si is not None and si.on_wait is not None and len(si.on_wait) > max_waits:
                    waits = list(si.on_wait)
                    extra, keep = waits[:-max_waits], waits[-max_waits:]
                    for w in extra:
                        nop = mybir.InstNoOp(
                            name=nc.get_next_instruction_name(),
                            text_hint="waitsplit",
                            bass_nofuse=True,
                        )
                        nop.engine = inst.engine
                        nop.sync_info = mybir.SyncInfo(on_wait=[w], on_update=[])
                        out.append(nop)
                        n_split += 1
                    si.on_wait = keep
                    inst.sync_info = si
                    changed = True
                out.append(inst)
            if changed:
                blk.instructions = out
    return n_split


def _build_program():
    nc = bass.Bass()
    x8 = nc.dram_tensor("x8", [C, T], FP8, kind="ExternalInput")
    r8 = nc.dram_tensor("r8", [C, T], FP8, kind="ExternalInput")
    setup = nc.dram_tensor("setup", [128, S_COLS], BF16, kind="ExternalInput")
    setup8 = nc.dram_tensor("setup8", [128, S8_COLS], FP8, kind="ExternalInput")
    setup8b = nc.dram_tensor("setup8b", [128, 512], FP8E5, kind="ExternalInput")
    setupf = nc.dram_tensor("setupf", [128, 2], F32, kind="ExternalInput")
    out = nc.dram_tensor("out", [128, NSL, 4, 65], F32, kind="ExternalOutput")

    # c = ((2d + u) * 128) + p  ->  [p, d, u, t]
    x8_v = x8[:].rearrange("(d u p) t -> p d u t", p=128, u=2)
    r8_v = r8[:].rearrange("(d u p) t -> p d u t", p=128, u=2)

    with TileContext(nc) as tc:
        with (
            tc.tile_pool(name="sb", bufs=1) as sb,
            tc.tile_pool(name="sbe2", bufs=18) as sbe2,
            tc.tile_pool(name="sbe1", bufs=9) as sbe1,
            tc.tile_pool(name="sbo", bufs=4) as sbo,
            tc.tile_pool(name="psProj", bufs=1, space="PSUM") as psProj,
            tc.tile_pool(name="psS1", bufs=1, space="PSUM") as psS1,
            tc.tile_pool(name="psS2", bufs=1, space="PSUM") as psS2,
            tc.tile_pool(name="psSg", bufs=1, space="PSUM") as psSg,
            tc.tile_pool(name="psO", bufs=2, space="PSUM") as psO,
        ):
            setup_sb = sb.tile([128, S_COLS], BF16)
            setup8_sb = sb.tile([128, S8_COLS], FP8)
            setup8b_sb = sb.tile([128, 512], FP8E5)
            setupf_sb = sb.tile([128, 2], F32)
            # qk weights first: they gate the first projection matmuls.
            # All setup DMAs issue from SP so the ACT sequencer stays free
            # for the psum exits.
            nc.sync.dma_start(out=setup8_sb[:, 0:1024],
                              in_=setup8[:, 0:1024])
            nc.sync.dma_start(out=setupf_sb[:], in_=setupf[:])
            nc.sync.dma_start(out=setup_sb[:], in_=setup[:])
            w8qk = setup8_sb[:, W8QK0:W8QK0 + 1024].rearrange(
                "p (d u m) -> p d u m", d=4, u=2)
            w8v = setup8_sb[:, W8V0:W8V0 + 512].rearrange(
                "p (d u m) -> p d u m", d=4, u=2)
            w8vr = setup8b_sb[:].rearrange("p (d u m) -> p d u m", d=4, u=2)

            tri = setup_sb[:, TRI0:TRI0 + 128].bitcast(F16)
            qk_bias = setupf_sb[:, 0:1]    # rows 64:127 = bq, rows 0:63 = 0
            qk_scale = setupf_sb[:, 1:2]   # rows 0:63 = 1/32, rows 64:127 = 1

            # warm the ACT exp table without waiting on any DMA
            warm = sb.tile([1, 2], F32)
            nc.vector.memset(warm[:, 0:1], 0.0)
            nc.scalar.activation(warm[:, 1:2], warm[:, 0:1], AF.Exp)

            x8_sb = sb.tile([128, 4, 2, T], FP8)
            r8_sb = sb.tile([128, 4, 2, T], FP8)
            import os as _os
            KDMA = _os.environ.get("KDMA", "inter")
            if KDMA == "xfirst":
                order = [("x", 0, 0), ("x", 0, 1), ("w", 0, 0), ("x", 1, None),
                         ("r", 0, None), ("x", 2, None), ("r", 1, None),
                         ("x", 3, None), ("r", 2, None), ("r", 3, None)]
            else:
                order = [("x", 0, 0), ("x", 0, 1), ("w", 0, 0), ("r", 0, None),
                         ("x", 1, None), ("r", 1, None), ("x", 2, None),
                         ("r", 2, None), ("x", 3, None), ("r", 3, None)]
            for kind, n, half in order:
                ts = slice(512 * n, 512 * (n + 1))
                if kind == "w":
                    nc.sync.dma_start(out=setup8_sb[:, 1024:S8_COLS],
                                      in_=setup8[:, 1024:S8_COLS])
                    nc.sync.dma_start(out=setup8b_sb[:], in_=setup8b[:])
                elif kind == "x":
                    if half is None:
                        nc.sync.dma_start(out=x8_sb[:, :, :, ts],
                                          in_=x8_v[:, :, :, ts])
                    else:
                        d = slice(2 * half, 2 * half + 2)
                        nc.sync.dma_start(out=x8_sb[:, d, :, ts],
                                          in_=x8_v[:, d, :, ts])
                else:
                    nc.sync.dma_start(out=r8_sb[:, :, :, ts],
                                      in_=r8_v[:, :, :, ts])

            # fp8 q/k for DoubleRow scores: k8 = k/8, q8 = (q+bq)/8, and the
            # stride-0 broadcast u-dim in the score matmul doubles the
            # product, giving the required 2*(1/8)*(1/8) = 1/32 score scale.
            kT_sb = sb.tile([64, T], FP8)      # k8 at parts 0:63
            qT_sb = sb.tile([64, T], FP8)      # q8 at parts 0:63
            v_sb = sb.tile([128, NQC, 65], F16)
            nc.vector.memset(v_sb[:, :, 64:65], 1.0)


            # running engine-busy estimates for greedy exp assignment
            load = {"a": 0.0, "v": 0.0}

            def proj_thunks(n):
                """Projection work for t-slice n as interleavable thunks."""
                ts = slice(512 * n, 512 * (n + 1))
                qk_ps = psProj.tile([128, 512], F32, tag="proj",
                                    name=f"qk{n}")
                thunks = []

                def qk_mm(d):
                    nc.tensor.matmul(
                        out=qk_ps[:],
                        lhsT=w8qk[:, d, :, :],
                        rhs=x8_sb[:, d, :, ts],
                        start=(d == 0), stop=(d == 3),
                        perf_mode=DR,
                    )
                for d in range(4):
                    thunks.append(lambda d=d: qk_mm(d))

                def k_exit():
                    # k8 = k/8 (psum parts 0:63 -> SBUF fp8)
                    load["a"] += 953.0
                    nc.scalar.activation(kT_sb[:, ts], qk_ps[0:64, :], AF.Copy,
                                         scale=0.125)

                def q_exit():
                    # q8 = (q + bq)/8 with the partition shift 64:127 -> 0:63
                    # done by the same DVE op (short latency; no DMA hop)
                    load["v"] += 833.0
                    nc.vector.tensor_scalar(
                        out=qT_sb[:, ts], in0=qk_ps[64:128, :],
                        scalar1=setupf_sb[64:128, 0:1], scalar2=0.125,
                        op0=ALU.add, op1=ALU.mult,
                    )
                thunks.append(k_exit)
                thunks.append(q_exit)
                vthunks = []

                vtile = []

                def v_mm(tch, k):
                    # 12 DoubleRow accumulation steps: x8@Wv8 + x8@Wvr8
                    # + r8@Wv8 (captures both x and Wv to second order)
                    if not vtile:
                        vtile.append(psProj.tile([128, 4, 64], F32,
                                                 tag="proj", name=f"v{n}"))
                    v_ps = vtile[0]
                    j = 4 * n + tch
                    tsj = slice(128 * j, 128 * (j + 1))
                    d = k % 4
                    p = k // 4
                    lhsT = (r8_sb if p == 2 else x8_sb)[:, d, :, tsj]
                    rhs = (w8vr if p == 1 else w8v)[:, d, :, :]
                    nc.tensor.matmul(
                        out=v_ps[:, tch, :],
                        lhsT=lhsT, rhs=rhs,
                        start=(k == 0), stop=(k == 11),
                        perf_mode=DR,
                        skip_group_check=True,
                    )
                for tch in range(4):
                    for k in range(12):
                        vthunks.append(lambda t=tch, k=k: v_mm(t, k))

                def v_exit():
                    if os.environ.get("KVEX", "v") == "v":
                        load["v"] += 566.0
                        nc.vector.tensor_copy(v_sb[:, 4 * n:4 * n + 4, 0:64],
                                              vtile[0][:])
                    else:
                        load["a"] += 727.0
                        nc.scalar.activation(v_sb[:, 4 * n:4 * n + 4, 0:64],
                                             vtile[0][:], AF.Copy)
                vthunks.append(v_exit)
                return thunks, vthunks

            e_tiles = {}
            slot_ctr = [0]
            last_eng = [""]
            pend = []   # [s_t, (I, p) of rel-0] when a tile is half filled

            def scores_tile(s_t, I, p, qs, rel):
                """Emit score matmuls for kv-pair tile p of super I into
                half `rel` of the 2-bank psum tile s_t."""
                for jj in range(2):
                    j = 2 * p + jj
                    if "noscores" not in ABL:
                        # fp8 DoubleRow with a stride-0 broadcast u-dim on
                        # both operands: computes 2*(k8^T @ q8) at 0.5
                        # cycles/row; the doubling is folded into the /8
                        # exit scales.
                        nc.tensor.matmul(
                            out=s_t[:, rel, jj, :],
                            lhsT=kT_sb[:, 128 * j:128 * (j + 1)]
                                .unsqueeze(1).broadcast_to([64, 2, 128]),
                            rhs=qT_sb[:, qs]
                                .unsqueeze(1).broadcast_to([64, 2, 256]),
                            start=True, stop=True,
                            perf_mode=DR,
                            skip_group_check=True,
                        )

            def exp_tile(s_t, halves):
                """Emit one exp op covering `halves` = [(I, p), ...] (1 or 2
                scores_tile units, possibly from different supers)."""
                pair = len(halves) == 2
                els = 1024 if pair else 512
                cost_a = els * 0.885 + 500.0
                cost_v = els * 1.04 + 150.0
                has_diag = any(p == I for (I, p) in halves)
                if "allact" in ABL or has_diag:
                    eng = "a"
                    load["a"] += cost_a
                elif KEXP == "alt":
                    eng = "a" if last_eng[0] != "a" else "v"
                    last_eng[0] = eng
                elif KEXP == "altr":
                    eng = "v" if last_eng[0] != "v" else "a"
                    last_eng[0] = eng
                elif KEXP == "altv":
                    import os as _o
                    pat = _o.environ.get("KPAT", "vva")
                    eng = pat[slot_ctr[0] % len(pat)]
                elif load["a"] + cost_a <= load["v"] + cost_v:
                    eng = "a" if last_eng[0] != "a" or \
                        load["v"] + cost_v - load["a"] - cost_a > 1200 else "v"
                    load[eng] += cost_a if eng == "a" else cost_v
                else:
                    eng = "v" if last_eng[0] != "v" or \
                        load["a"] + cost_a - load["v"] - cost_v > 1200 else "a"
                    load[eng] += cost_a if eng == "a" else cost_v
                last_eng[0] = eng
                I0, q0 = halves[0]
                if pair:
                    src = s_t[:]
                    e_t = sbe2.tile([128, 2, 2, 256], F16, tag="e2",
                                    name=f"e{I0}_{q0}")
                else:
                    src = s_t[:, 0, :, :]
                    e_t = sbe1.tile([128, 2, 256], F16, tag="e1",
                                    name=f"e{I0}_{q0}")
                if "noexp" in ABL:
                    nc.vector.memset(e_t[0:1, 0, 0:1] if not pair else e_t[0:1, 0, 0, 0:1], 1.0)
                elif eng == "a":
                    nc.scalar.activation(e_t[:], src, AF.Exp)
                else:
                    nc.vector.tensor_scalar(
                        out=e_t[:].bitcast(I16), in0=src,
                        scalar1=float(EXP_A), scalar2=float(EXP_B),
                        op0=ALU.mult, op1=ALU.add,
                    )
                for rel, (I, pd) in enumerate(halves):
                    e_tiles[(I, pd)] = (e_t, rel if pair else None)
                    if pd == I:
                        # diagonal blocks: zero above-diagonal probabilities
                        # on the idle gpsimd engine (SBUF-only)
                        for jj in range(2):
                            if pair:
                                blk = e_t[:, rel, jj, 128 * jj:128 * (jj + 1)]
                            else:
                                blk = e_t[:, jj, 128 * jj:128 * (jj + 1)]
                            if "nomask" not in ABL:
                                nc.gpsimd.tensor_tensor(out=blk, in0=blk,
                                                        in1=tri, op=ALU.mult)

            def pv_thunks(I, o_ps):
                """PV matmul thunks for super I, chunk-major (accumulation
                groups per o bank must be strictly sequential)."""
                thunks = []
                for ii in range(2):
                    i = 2 * I + ii
                    oc = i % 4
                    for j in range(i + 1):
                        p = j // 2

                        def mk(i=i, j=j, p=p, oc=oc):
                            e_t, rel = e_tiles[(I, p)]
                            if rel is not None:
                                lhsT = e_t[:, rel, j % 2,
                                           128 * (i % 2):128 * (i % 2) + 128]
                            else:
                                lhsT = e_t[:, j % 2,
                                           128 * (i % 2):128 * (i % 2) + 128]
                            if "nopv" not in ABL:
                                nc.tensor.matmul(
                                    out=o_ps[:, oc, :],
                                    lhsT=lhsT,
                                    rhs=v_sb[:, j, :],
                                    start=(j == 0), stop=(j == i),
                                    skip_group_check=True,
                                )
                        thunks.append(mk)
                return thunks

            def out_stage(m, o_ps, half=None):
                ob = ob_tiles.get(m)
                if ob is None:
                    ob = sbo.tile([128, 4, 65], F32, tag="ob", name=f"ob{m}")
                    ob_tiles[m] = ob
                if half is None:
                    h = slice(0, 4)
                elif half in (0, 1):
                    h = slice(2 * half, 2 * half + 2)
                else:
                    h = slice(half - 2, half - 1)   # single chunk column
                if KOUT == "a":
                    load["a"] += 730.0
                    nc.scalar.activation(ob[:, h, :], o_ps[:, h, :], AF.Copy)
                else:
                    load["v"] += 571.0
                    nc.vector.tensor_copy(ob[:, h, :], o_ps[:, h, :])
                eng = nc.scalar if m == NSL - 1 else nc.gpsimd
                eng.dma_start(out=out[:, m, h, :], in_=ob[:, h, :])

            ob_tiles = {}

            o_tiles = {}

            def lagged_work(I):
                """PV thunks for super I-1 (+ out stage), to interleave with
                super I's score/exp emission."""
                if I < 1 or I > NSUP:
                    return []
                Ip = I - 1
                m = Ip // 2
                if Ip % 2 == 0:
                    o_tiles[m] = psO.tile([128, 4, 65], F32, tag="o",
                                          name=f"o{m}")
                thunks = pv_thunks(Ip, o_tiles[m])
                if Ip == NSUP - 2:
                    # chunks 12,13 are finished after super 6's PV: ship that
                    # half early so only chunks 14,15 remain for the tail
                    thunks.append(lambda: out_stage(NSL - 1, o_tiles[m], 0))
                elif Ip % 2 == 1:
                    if Ip == NSUP - 1:
                        thunks.append(lambda: out_stage(m, o_tiles[m], 1))
                    else:
                        thunks.append(lambda: out_stage(m, o_tiles[m]))
                return thunks

            def super_block(I, extra=()):
                """Scores+exp for super I with super I-1's PV (and the next
                slice's projection) interleaved so the PE always has filler
                while the exp engines drain psum tiles."""
                qs = slice(256 * I, 256 * (I + 1))
                lag = list(extra) + lagged_work(I)
                pairs = list(range(0, I + 1, 2))
                li = 0
                for u, p0 in enumerate(pairs):
                    pair = p0 + 1 <= I
                    if pair:
                        pool = (psS1, psS2)[slot_ctr[0] % 2]
                        slot_ctr[0] += 1
                        s_t = pool.tile([128, 2, 2, 256], F32, tag="s",
                                        name=f"s{I}_{p0}")
                    else:
                        s_t = psSg.tile([128, 1, 2, 256], F32, tag="sg",
                                        name=f"s{I}_{p0}")
                    scores_tile(s_t, I, p0, qs, 0)
                    if pair:
                        scores_tile(s_t, I, p0 + 1, qs, 1)
                        exp_tile(s_t, [(I, p0), (I, p0 + 1)])
                    else:
                        exp_tile(s_t, [(I, p0)])
                    if KLAG == "prop":
                        goal = len(lag) * (u + 1) // len(pairs)
                        while li < goal:
                            lag[li]()
                            li += 1
                while li < len(lag):
                    lag[li]()
                    li += 1

            for n in range(NSL):
                qk_th, v_th = proj_thunks(n)
                for th in qk_th:
                    th()
                for th in v_th:
                    th()
                super_block(2 * n)
                super_block(2 * n + 1)
            for th in lagged_work(NSUP):
                th()

    _split_waits(nc)
    return nc


def _dr_layout(w8):
    """[C, M] -> DoubleRow lhsT/rhs layout [128, 4*2*M] (c = 256d+128u+p)."""
    M = w8.shape[1]
    return np.ascontiguousarray(
        w8.reshape(4, 2, 128, M).transpose(2, 0, 1, 3).reshape(128, 8 * M)
    )


def _make_setup(Wq, Wk, Wv):
    FP8NP = ml_dtypes.float8_e4m3
    # bf16 setup: just the causal mask, stored as fp16 bit patterns
    bits = np.zeros((128, S_COLS), dtype=np.uint16)
    bits[:, TRI0:TRI0 + 128] = np.triu(np.ones((128, 128), np.float16)).view(
        np.uint16)
    setup = bits.view(ml_dtypes.bfloat16)

    s8 = np.zeros((128, S8_COLS), dtype=FP8NP)
    wqk = np.concatenate([Wk, Wq], axis=1)                     # [C, 128]
    s8[:, W8QK0:W8QK0 + 1024] = _dr_layout(wqk.astype(FP8NP))
    wv8 = Wv.astype(FP8NP)
    s8[:, W8V0:W8V0 + 512] = _dr_layout(wv8)
    s8b = _dr_layout(
        (Wv - wv8.astype(np.float32)).astype(ml_dtypes.float8_e5m2))
    return setup, s8, s8b


_PROGRAM = None
_RUNNER = None


def _make_runner():
    """Build the SPMD jitted executable once (mirrors bass2jax.run_bass_via_pjrt,
    but cached so repeat calls skip retracing/XLA recompile)."""
    import jax
    import numpy as _np
    from jax.experimental.shard_map import shard_map
    from jax.sharding import Mesh, PartitionSpec
    from concourse import bass2jax

    nc = _PROGRAM
    bass2jax.install_neuronx_cc_hook()
    import concourse.mybir as _mybir

    in_names, out_names, out_avals, zero_outs = [], [], [], []
    for alloc in nc.m.functions[0].allocations:
        if not isinstance(alloc, _mybir.MemoryLocationSet):
            continue
        name = alloc.memorylocations[0].name
        pname = nc.partition_id_tensor.name if nc.partition_id_tensor else None
        if alloc.kind == "ExternalInput":
            if name != pname:
                in_names.append(name)
        elif alloc.kind == "ExternalOutput":
            shape = tuple(alloc.tensor_shape)
            dtype = _mybir.dt.np(alloc.dtype)
            out_names.append(name)
            out_avals.append(jax.core.ShapedArray(shape, dtype))
            zero_outs.append(_np.zeros(shape, dtype))
    n_params = len(in_names)
    n_outs = len(out_avals)
    all_names = in_names + out_names
    if nc.partition_id_tensor is not None:
        all_names = all_names + [nc.partition_id_tensor.name]

    def _body(*args):
        operands = list(args)
        if nc.partition_id_tensor is not None:
            operands.append(bass2jax.partition_id_tensor())
        outs = bass2jax._bass_exec_p.bind(
            *operands,
            out_avals=tuple(out_avals),
            in_names=tuple(all_names),
            out_names=tuple(out_names),
            lowering_input_output_aliases=(),
            sim_require_finite=True,
            sim_require_nnan=True,
            nc=nc,
        )
        return tuple(outs)

    devices = jax.devices()[:NCORES]
    mesh = Mesh(_np.asarray(devices), ("core",))
    in_specs = (PartitionSpec("core"),) * (n_params + n_outs)
    out_specs = (PartitionSpec("core"),) * n_outs
    sharded = jax.jit(
        shard_map(_body, mesh=mesh, in_specs=in_specs, out_specs=out_specs,
                  check_rep=False),
        donate_argnums=tuple(range(n_params, n_params + n_outs)),
        keep_unused=True,
    )

    def run(in_maps):
        concat_in = [
            _np.concatenate([in_maps[c][name] for c in range(NCORES)], axis=0)
            for name in in_names
        ]
        concat_zero = [
            _np.concatenate([z] * NCORES, axis=0) for z in zero_outs
        ]
        outs = sharded(*concat_in, *concat_zero)
        res = []
        for c in range(NCORES):
            m = {}
            for i, name in enumerate(out_names):
                per = _np.split(_np.asarray(outs[i]), NCORES, axis=0)
                m[name] = per[c]
            res.append(m)
        return res

    return run


def kernel(x, Wq, bq, Wk, bk, Wv, bv):
    global _PROGRAM, _RUNNER
    x = np.asarray(x, dtype=np.float32)
    if _PROGRAM is None:
        _PROGRAM = _build_program()
    if _RUNNER is None:
        try:
            _RUNNER = _make_runner()
        except Exception:
            def _RUNNER(in_maps):
                return run_bass_kernel_spmd(
                    _PROGRAM, in_maps, core_ids=list(range(NCORES))
                ).results
    setup, setup8, setup8b = _make_setup(
        np.asarray(Wq, np.float32), np.asarray(Wk, np.float32),
        np.asarray(Wv, np.float32),
    )
    # bk is softmax-invariant (constant along each score row) and dropped;
    # bq is folded into the fused qk ACT exit; bv is added on the host.
    setupf = np.zeros((128, 2), np.float32)
    setupf[64:128, 0] = np.asarray(bq, np.float32)
    FP8NP = ml_dtypes.float8_e4m3
    in_maps = []
    for b in range(NCORES):
        xT = np.ascontiguousarray(x[b].T)
        x8b = xT.astype(FP8NP)
        r8b = (xT - x8b.astype(np.float32)).astype(FP8NP)
        in_maps.append({
            "x8": x8b,
            "r8": r8b,
            "setup": setup,
            "setup8": setup8,
            "setup8b": setup8b,
            "setupf": setupf,
        })
    res = _RUNNER(in_maps)
    bvf = np.asarray(bv, np.float32)
    outs = []
    for b in range(NCORES):
        o = np.asarray(res[b]["out"], np.float32)      # [128, 4, 4, 65]
        num = o[:, :, :, 0:64]
        den = o[:, :, :, 64:65]
        o = num / den                                   # [128, 16 chunks, 64]
        o = o.reshape(128, NQC, H).transpose(1, 0, 2).reshape(T, H) + bvf
        outs.append(o)
    return np.stack(outs)

